# revision 1
# baseline (speedup 1.0000x reference)
"""LoRA MLP (2->64->64->64->64->64->3, tanh) over N=1,048,576 rows.

Strategy:
  - Host: merge LoRA into dense weights (W_eff = W + B@A), build
    block-diagonal lhsT so each 128-wide PE pass processes TWO row-chunks
    (features of chunk A on partitions 0..63, chunk B on 64..127).
  - 8 cores, pure data parallel: 131072 rows/core = 65536 columns
    (each SBUF column carries one row of chunk A and one row of chunk B).
  - Per 2048-col block: 4 fp16 matmuls (full-rate on the PE) into a
    4-bank fp32 PSUM tile, one [128,2048] ACT tanh with fused fp32
    per-partition bias, last layer bias-add on DVE. tanh on the scalar
    engine is the bottleneck; PE/DVE/DMA hide underneath it.
  - fp16 end-to-end numerics emulated on host: max scale-relative error
    ~1.1e-3 vs the fp32 reference (bf16 would be ~8e-3).
  - x stays fully SBUF-resident (one DMA); outputs stream back per block.
"""

import numpy as np
from contextlib import ExitStack

import concourse.bacc as bacc
import concourse.tile as tile
from concourse import mybir
from concourse.bass_utils import run_bass_kernel_spmd

N = 1_048_576
NCORES = 8
N_CORE = N // NCORES          # 131072 rows per core
NCOLS = N_CORE // 2           # 65536 cols (2 rows per col: chunk A + chunk B)
BLK = 2048                    # columns per block (PSUM tile = 4 banks)
NBLK = NCOLS // BLK           # 32 blocks
MM = 512                      # moving free dim per matmul (1 PSUM bank)
WB_COLS = 1798                # packed fp16 weights + bias rows + ones
W_DVE = 0                     # columns per tile handled by the DVE tanh poly

F32 = mybir.dt.float32
F16 = mybir.dt.float16

# Degree-9 odd polynomial for tanh on [-L, L] (DVE offload path):
# y = x * (A t^4 + (B+C) t^3 + D t^2 + E t + F), t = (x/L)^2, |err| <= 1.4e-4
TANH_L = 1.6
TANH_S = float(np.float32(1.0 / (TANH_L * TANH_L)))
TANH_A = 0.11883190274238586
TANH_B = -0.4309496581554413
TANH_C = 0.0
TANH_D = 0.7199327945709229
TANH_E = -0.8307216167449951
TANH_F = 0.9990311861038208


# Set by the last kernel() call (profiling info for test.py).
LAST_RESULT = None


def build_nc(repeat=1):
    nc = bacc.Bacc(None, target_bir_lowering=False)

    xt = nc.dram_tensor("xt", [4, NCOLS], F16, kind="ExternalInput")
    wb = nc.dram_tensor("wb", [128, WB_COLS], F16, kind="ExternalInput")
    bias = nc.dram_tensor("bias", [128, 6], F32, kind="ExternalInput")
    out_t = nc.dram_tensor("out_t", [6, NCOLS], F16, kind="ExternalOutput")

    with tile.TileContext(nc) as tc, ExitStack() as ctx:
        const = ctx.enter_context(tc.tile_pool(name="const", bufs=1))
        h_pool = ctx.enter_context(tc.tile_pool(name="h", bufs=6))
        o_pool = ctx.enter_context(tc.tile_pool(name="o", bufs=2))
        dve_pool = ctx.enter_context(tc.tile_pool(name="dve", bufs=2))
        ps_pool = ctx.enter_context(tc.tile_pool(name="ps", bufs=2, space="PSUM"))

        wb_sb = const.tile([128, WB_COLS], F16, tag="wb")
        nc.gpsimd.dma_start(out=wb_sb, in_=wb[:, :])
        bias_sb = const.tile([128, 6], F32, tag="bias")
        nc.gpsimd.dma_start(out=bias_sb, in_=bias[:, :])

        # whole per-core x resident in SBUF: one DMA, no slot reuse
        xfull = const.tile([4, NCOLS], F16, tag="xfull")
        XCH = NCOLS // 8
        for ch in range(8):
            nc.gpsimd.dma_start(
                out=xfull[:, ch * XCH : (ch + 1) * XCH],
                in_=xt[:, ch * XCH : (ch + 1) * XCH],
            )

        # lhsT views: layer1 [4,128] at cols 512..639 (rows 0..3),
        # layers 2..5 [128,128] at cols 0..511, layer6 [128,6] at 640..645
        w_sb = [wb_sb[0:4, 512:640]]
        for i in range(4):
            w_sb.append(wb_sb[:, i * 128 : (i + 1) * 128])
        w_sb.append(wb_sb[:, 640:646])
        b_sb = [bias_sb[:, i : i + 1] for i in range(5)]
        b_sb.append(bias_sb[0:6, 5:6])
        # bias as [1,128] rows (for the PE ones-trick on the DVE slice)
        brow_sb = [wb_sb[0:1, 646 + 128 * i : 774 + 128 * i] for i in range(5)]
        ones_sb = wb_sb[0:1, 1286:1798]  # 512 ones

        # Two chains (first/second half of the blocks) run interleaved
        # with a 3-layer phase stagger: when one chain is in its layer-6
        # epilogue (matmul + DVE bias-add holding a psum slot), the other
        # is mid-tanh, so the scalar engine never starves.
        halves = NBLK // 2
        steps = halves * 6
        SHIFT = 3
        hh = [None, None]

        last_ps = [None, None]
        tanh_count = [0]

        def dve_tanh(hn, ps, s0, w):
            # tanh on ps[:, s0:s0+w] (bias already accumulated in psum)
            op = mybir.AluOpType
            zsl = ps[:, s0 : s0 + w]
            xc_t = dve_pool.tile([128, W_DVE], F16, tag="xc")
            xc = xc_t[:, 0:w]
            nc.vector.tensor_scalar(
                out=xc, in0=zsl, scalar1=-TANH_L, scalar2=TANH_L,
                op0=op.max, op1=op.min,
            )
            tt_t = dve_pool.tile([128, W_DVE], F16, tag="tt")
            tt = tt_t[:, 0:w]
            nc.vector.scalar_tensor_tensor(
                out=tt, in0=xc, scalar=TANH_S, in1=xc, op0=op.mult, op1=op.mult
            )
            u_t = dve_pool.tile([128, W_DVE], F16, tag="u")
            u = u_t[:, 0:w]
            nc.vector.tensor_scalar(
                out=u, in0=tt, scalar1=TANH_A, scalar2=TANH_B,
                op0=op.mult, op1=op.add,
            )
            for g in (TANH_C, TANH_D, TANH_E):
                nc.vector.scalar_tensor_tensor(
                    out=u, in0=u, scalar=g, in1=tt, op0=op.add, op1=op.mult
                )
            nc.vector.scalar_tensor_tensor(
                out=hn[:, s0 : s0 + w], in0=u, scalar=TANH_F, in1=xc,
                op0=op.add, op1=op.mult,
            )

        def emit_step(chain, idx, rep):
            b = idx // 6
            layer = idx % 6
            blk = chain * halves + b
            c0 = blk * BLK
            if layer == 0:
                hh[chain] = xfull[:, c0 : c0 + BLK]
            h = hh[chain]
            if layer < 5:
                ps = ps_pool.tile([128, BLK], F32, tag="ps")
                po = ps[:, :]
                last_ps[chain] = ps
            else:
                # layer 6 reuses the L5 psum tile (partitions 0:6) after
                # the tanh has read it - saves a psum slot allocation
                ps = last_ps[chain]
                po = ps[0:6, :]
            nq = BLK // MM
            for q in range(nq):
                biasq = layer < 5 and W_DVE > 0 and q == nq - 1
                nc.tensor.matmul(
                    out=po[:, q * MM : (q + 1) * MM],
                    lhsT=w_sb[layer],
                    rhs=h[:, q * MM : (q + 1) * MM],
                    start=True,
                    stop=not biasq,
                )
                if biasq:
                    # accumulate bias over the DVE slice via a ones row
                    nc.tensor.matmul(
                        out=po[:, BLK - W_DVE : BLK],
                        lhsT=brow_sb[layer],
                        rhs=ones_sb[0:1, 0:W_DVE],
                        start=False,
                        stop=True,
                    )
            if layer < 5:
                hn = h_pool.tile([128, BLK], F16, tag="h")
                nc.scalar.activation(
                    out=hn[:, 0 : BLK - W_DVE],
                    in_=ps[:, 0 : BLK - W_DVE],
                    func=mybir.ActivationFunctionType.Tanh,
                    bias=b_sb[layer],
                )
                if W_DVE > 0:
                    dve_tanh(hn, ps, BLK - W_DVE, W_DVE)
                hh[chain] = hn
            else:
                ot = o_pool.tile([6, BLK], F16, tag="o")
                nc.vector.tensor_scalar_add(ot[:, :], ps[0:6, :], b_sb[5])
                nc.gpsimd.dma_start(out=out_t[:, c0 : c0 + BLK], in_=ot)

        for rep in range(repeat):
            for i in range(steps + SHIFT):
                if i < steps:
                    emit_step(0, i, rep)
                j = i - SHIFT
                if 0 <= j < steps:
                    emit_step(1, j, rep)

    nc.compile()
    return nc


def _prep_weights(inputs):
    """Merged LoRA weights (fp16, block-diagonal lhsT) + fp32 biases."""

    def eff(w, bmat, amat):
        return (
            w.astype(np.float64) + bmat.astype(np.float64) @ amat.astype(np.float64)
        ).astype(np.float32)

    wb = np.zeros((128, WB_COLS), np.float16)
    for i in (2, 3, 4, 5):
        wl = eff(inputs[f"W{i}"], inputs[f"B{i}"], inputs[f"A{i}"])  # [64, 64]
        c = (i - 2) * 128
        wb[0:64, c : c + 64] = wl.T.astype(np.float16)
        wb[64:128, c + 64 : c + 128] = wl.T.astype(np.float16)
    w1 = eff(inputs["W1"], inputs["B1"], inputs["A1"])  # [64, 2]
    wb[0:2, 512:576] = w1.T.astype(np.float16)
    wb[2:4, 576:640] = w1.T.astype(np.float16)
    w6 = eff(inputs["W6"], inputs["B6"], inputs["A6"])  # [3, 64]
    wb[0:64, 640:643] = w6.T.astype(np.float16)
    wb[64:128, 643:646] = w6.T.astype(np.float16)

    for i in (1, 2, 3, 4, 5):
        b = np.asarray(inputs[f"b{i}"], np.float32).reshape(64)
        wb[0, 646 + 128 * (i - 1) : 710 + 128 * (i - 1)] = b.astype(np.float16)
        wb[0, 710 + 128 * (i - 1) : 774 + 128 * (i - 1)] = b.astype(np.float16)
    wb[0, 1286:1798] = np.float16(1.0)

    bias = np.zeros((128, 6), np.float32)
    for i in (1, 2, 3, 4, 5):
        b = np.asarray(inputs[f"b{i}"], np.float32).reshape(64)
        bias[:, i - 1] = np.concatenate([b, b])
    b6 = np.asarray(inputs["b6"], np.float32).reshape(3)
    bias[0:3, 5] = b6
    bias[3:6, 5] = b6
    return {"wb": wb, "bias": bias}


def kernel(**inputs):
    global LAST_RESULT
    inputs = {k: np.asarray(v, np.float32) for k, v in inputs.items()}
    ws = _prep_weights(inputs)

    x = inputs["x"]  # [N, 2]
    in_maps = []
    for c in range(NCORES):
        sh = x[c * N_CORE : (c + 1) * N_CORE]  # [131072, 2]
        xtc = np.empty((4, NCOLS), np.float16)
        xtc[0:2] = sh[:NCOLS].T
        xtc[2:4] = sh[NCOLS:].T
        m = {"xt": np.ascontiguousarray(xtc)}
        m.update(ws)
        in_maps.append(m)

    nc = build_nc()
    res = run_bass_kernel_spmd(nc, in_maps, core_ids=list(range(NCORES)))
    LAST_RESULT = res

    u = np.empty((N, 1), np.float32)
    v = np.empty((N, 1), np.float32)
    w = np.empty((N, 1), np.float32)
    for c in range(NCORES):
        o = res.results[c]["out_t"]  # [6, NCOLS] fp16
        base = c * N_CORE
        u[base : base + NCOLS, 0] = o[0]
        v[base : base + NCOLS, 0] = o[1]
        w[base : base + NCOLS, 0] = o[2]
        u[base + NCOLS : base + N_CORE, 0] = o[3]
        v[base + NCOLS : base + N_CORE, 0] = o[4]
        w[base + NCOLS : base + N_CORE, 0] = o[5]
    return (u, v, w)


def measure_exec_ns(r=17, rounds=12):
    """Per-execution HW time via paired repeat-delta (drift-immune): the
    same inputs run through a 1x and an r-x internally-repeated build,
    alternating per round; per-exec = median(t_r - t_1) / (r - 1)."""
    import time as _time

    import jax
    from jax.sharding import Mesh, PartitionSpec
    from jax.experimental.shard_map import shard_map

    from concourse.bass2jax import (
        _bass_exec_p,
        install_neuronx_cc_hook,
        partition_id_tensor,
    )

    z_in = np.load("ref_cache.npz")
    inputs = {k[3:]: np.asarray(z_in[k], np.float32)
              for k in z_in.files if k.startswith("in_")}
    ws = _prep_weights(inputs)
    x = inputs["x"]
    in_maps = []
    for c in range(NCORES):
        sh = x[c * N_CORE : (c + 1) * N_CORE]
        xtc = np.empty((4, NCOLS), np.float16)
        xtc[0:2] = sh[:NCOLS].T
        xtc[2:4] = sh[NCOLS:].T
        m = {"xt": np.ascontiguousarray(xtc)}
        m.update(ws)
        in_maps.append(m)

    def make_fn(nc):
        install_neuronx_cc_hook()
        in_names, out_names, out_avals = [], [], []
        for alloc in nc.m.functions[0].allocations:
            if not isinstance(alloc, mybir.MemoryLocationSet):
                continue
            name = alloc.memorylocations[0].name
            if alloc.kind == "ExternalInput":
                in_names.append(name)
            elif alloc.kind == "ExternalOutput":
                out_names.append(name)
                out_avals.append(jax.core.ShapedArray(
                    tuple(alloc.tensor_shape), mybir.dt.np(alloc.dtype)))
        pname = nc.partition_id_tensor.name if nc.partition_id_tensor else None
        if pname in in_names:
            in_names.remove(pname)
        all_in = in_names + out_names + ([pname] if pname else [])

        def _body(*flat):
            extra = (partition_id_tensor(),) if pname else ()
            return tuple(_bass_exec_p.bind(
                *flat, *extra, out_avals=tuple(out_avals),
                in_names=tuple(all_in), out_names=tuple(out_names),
                lowering_input_output_aliases=(), sim_require_finite=True,
                sim_require_nnan=True, nc=nc))

        mesh = Mesh(np.asarray(jax.devices()[:NCORES]), ("core",))
        specs = (PartitionSpec("core"),) * (len(in_names) + len(out_names))
        f = jax.jit(shard_map(_body, mesh=mesh, in_specs=specs,
                    out_specs=(PartitionSpec("core"),) * len(out_names),
                    check_rep=False), keep_unused=True)
        return f, in_names

    mesh = Mesh(np.asarray(jax.devices()[:NCORES]), ("core",))
    sharding = jax.sharding.NamedSharding(mesh, PartitionSpec("core"))
    variants = []
    for rep in (1, r):
        f, in_names = make_fn(build_nc(repeat=rep))
        per_core = [[np.asarray(m[nm]) for nm in in_names] for m in in_maps]
        concat = [np.concatenate([per_core[c][i] for c in range(NCORES)], axis=0)
                  for i in range(len(in_names))]
        concat.append(np.zeros((NCORES * 6, NCOLS), np.float16))
        dev = [jax.device_put(a, sharding) for a in concat]
        jax.block_until_ready(dev)
        jax.block_until_ready(f(*dev))
        variants.append((f, dev))
    deltas = []
    for _ in range(rounds):
        ts = []
        for f, dev in variants:
            t0 = _time.time()
            jax.block_until_ready(f(*dev))
            ts.append(_time.time() - t0)
        deltas.append(ts[1] - ts[0])
    deltas.sort()
    return deltas[len(deltas) // 2] / (r - 1) * 1e9



# revision 9
# speedup vs baseline: 1.8301x; 1.8301x over previous
"""LoRA MLP (2->64->64->64->64->64->3, tanh) over N=1,048,576 rows.

Strategy (v2):
  - Host: merge LoRA into dense weights (W_eff = W + B@A), build
    block-diagonal lhsT so each 128-wide PE pass processes TWO row-chunks
    (features of chunk A on partitions 0..63, chunk B on 64..127).
  - 8 cores, pure data parallel: 131072 rows/core = 65536 columns.
  - tanh is split across BOTH the scalar (ACT) engine and the vector
    (DVE) engine: a custom fused DVE op evaluates a per-layer degree-5
    odd minimax polynomial x*(c0 + c1 x^2 + c2 x^4) (with the fused
    per-partition bias add) in ONE DVE instruction at 1 elem/lane/cyc,
    nearly matching ACT's tanh throughput. Blocks of 1024 columns are
    statically assigned to ACT-chains or DVE-chains so both engines run
    ~100% busy; the PE (6 matmul passes) runs just below them.
  - Layer-6 outputs ([6,1024] per block) are batched 4 blocks per PSUM
    tile at PE quadrant offsets 0/32/64/96, then flushed with a single
    [128,1024] copy (alternating ACT/DVE) -> SBUF -> DMA. b6 is added
    on the host (free).
  - PSUM: 3x [128,1024] main tiles (6 banks) + 1x [128,1024] group tile
    (2 banks) = 8 banks.
  - Polynomial coefficients are fit at runtime from a sampled forward
    pass (per-layer preactivation range), so accuracy tracks the data.
"""

import os
import numpy as np
from contextlib import ExitStack

import concourse.bacc as bacc
import concourse.tile as tile
from concourse import mybir
from concourse.bass_utils import run_bass_kernel_spmd

import concourse.dve_ops as _dve_ops
from concourse.dve_spec import (
    Spec, Src0, C0, C1, C2, C3, sq, lower, _spill_c3_to_src1,
)
from concourse.dve_ops import DveOp, OPS, CUSTOM_DVE_SPECS
from concourse.dve_uop import DveOpSpec

N = 1_048_576
NCORES = 8
N_CORE = N // NCORES          # 131072 rows per core
NCOLS = N_CORE // 2           # 65536 cols (2 rows per col: chunk A + chunk B)
BLK = 1024                    # columns per block (psum tile = 2 banks)
NBLK = NCOLS // BLK           # 64 blocks
MM = 512                      # moving free dim per matmul (1 PSUM bank)
N_D = int(os.environ.get("BASS_ND", "39"))  # blocks w/ layers 2-5 on DVE poly
GRP = 4                       # layer-6 outputs grouped per psum tile

F32 = mybir.dt.float32
F16 = mybir.dt.float16

# Set by the last kernel() call (profiling info for test.py).
LAST_RESULT = None


# ---------------------------------------------------------------------------
# Custom fused DVE op: out = (((c2*t + c1)*t + c0) * x, x = in0 + bias,
# t = x*x.  One DVE instruction per [128, BLK] tile (1 elem/lane/cyc).
# ---------------------------------------------------------------------------

def _tanh5_ref(in0, in1, s0, s1, imm2):
    x = in0.astype(np.float32) + s0
    t = x * x
    return ((s1 * t + imm2) * t + in1) * x


def _register_tanh5():
    for op in OPS:
        if op.name == "TANH5_ANT":
            return op
    _x = Src0 + C0
    _t = sq(_x)
    body = _spill_c3_to_src1((((C1 * _t) + C2) * _t + C3) * _x)
    spec = Spec(body=body, reference=_tanh5_ref)
    shas = {}
    for ver in ("v3", "v4"):
        shas[ver] = DveOpSpec(
            name="TANH5_ANT", opcode=1 + len(OPS),
            uops=lower(spec, ver=ver), rd1_en=True,
        ).sha(ver)
    op = DveOp("TANH5_ANT", spec, subdim=False, uops_sha=shas)
    OPS.append(op)
    CUSTOM_DVE_SPECS[op.name] = op.spec
    _dve_ops._SUB_OPCODE_FOR_NAME[op.name] = OPS.index(op) + 1
    return op


TANH5 = _register_tanh5()


# ---------------------------------------------------------------------------
# Per-layer degree-5 odd minimax fit of tanh on [0, B] (Lawson iteration).
# ---------------------------------------------------------------------------

def _fit_tanh5(B, n=4001, iters=40):
    xs = np.linspace(0.0, B, n)
    y = np.tanh(xs)
    A = np.stack([xs, xs**3, xs**5], axis=1)
    w = np.ones(n)
    c = None
    for _ in range(iters):
        Aw = A * w[:, None]
        c, *_ = np.linalg.lstsq(Aw, y * w, rcond=None)
        e = np.abs(A @ c - y)
        w *= e + 1e-12
        w /= w.max()
    return c.astype(np.float64)  # (c0, c1, c2)


def _poly_coeffs(inputs, Weff, beff):
    """Sampled forward pass -> per-layer preactivation range -> coeffs."""
    xs = np.asarray(inputs["x"], np.float32)[::16].astype(np.float32)
    h = xs
    coeffs = []
    for l in range(5):
        z = h @ Weff[l].T.astype(np.float32) + beff[l].astype(np.float32)
        B = float(np.abs(z).max()) * 1.08 + 0.03
        c = _fit_tanh5(B)
        coeffs.append(c)
        h = np.tanh(z)
    return coeffs  # list of (c0, c1, c2)


# ---------------------------------------------------------------------------
# Kernel build
# ---------------------------------------------------------------------------

WB_COLS = 646  # packed fp16 lhsT weights


def _d_blocks():
    picks = sorted({int(round(i * NBLK / N_D)) for i in range(N_D)})
    # de-dup & fill to exactly N_D entries
    out, used = [], set()
    for p in picks:
        while p in used:
            p += 1
        p %= NBLK
        while p in used:
            p = (p + 1) % NBLK
        used.add(p)
        out.append(p)
    return sorted(out)


def build_nc(coeffs, repeat=1):
    nc = bacc.Bacc(None, target_bir_lowering=False)

    xt = nc.dram_tensor("xt", [4, NCOLS], F16, kind="ExternalInput")
    wb = nc.dram_tensor("wb", [128, WB_COLS], F16, kind="ExternalInput")
    biasc = nc.dram_tensor("biasc", [128, 10], F32, kind="ExternalInput")
    out_t = nc.dram_tensor("out_t", [6, NCOLS], F16, kind="ExternalOutput")

    dset = set(_d_blocks())
    a_blocks = [b for b in range(NBLK) if b not in dset]
    d_blocks = [b for b in range(NBLK) if b in dset]

    # chains: two ACT chains, two DVE chains (interleave to hide deps)
    chains = [
        {"kind": "A", "blocks": a_blocks[0::2]},
        {"kind": "A", "blocks": a_blocks[1::2]},
        {"kind": "D", "blocks": d_blocks[0::2]},
        {"kind": "D", "blocks": d_blocks[1::2]},
    ]
    CA, CD, CL6 = 1150.0, 1200.0, 500.0

    def step_cost(chain, step):
        layer = step % 6
        if layer == 5:
            return CL6
        if chain["kind"] == "A" or layer == 0:
            return CA
        return CD

    with tile.TileContext(nc) as tc, ExitStack() as ctx:
        const = ctx.enter_context(tc.tile_pool(name="const", bufs=1))
        h_pool = ctx.enter_context(tc.tile_pool(name="h", bufs=3))
        gs_pool = ctx.enter_context(tc.tile_pool(name="gs", bufs=2))
        ps_pool = ctx.enter_context(tc.tile_pool(name="ps", bufs=3, space="PSUM"))
        pg_pool = ctx.enter_context(tc.tile_pool(name="pg", bufs=1, space="PSUM"))

        wb_sb = const.tile([128, WB_COLS], F16, tag="wb")
        nc.gpsimd.dma_start(out=wb_sb, in_=wb[:, :])
        bias_sb = const.tile([128, 10], F32, tag="biasc")
        nc.gpsimd.dma_start(out=bias_sb, in_=biasc[:, :])

        xfull = const.tile([4, NCOLS], F16, tag="xfull")
        XCH = NCOLS // 8
        for ch in range(8):
            nc.gpsimd.dma_start(
                out=xfull[:, ch * XCH : (ch + 1) * XCH],
                in_=xt[:, ch * XCH : (ch + 1) * XCH],
            )

        # lhsT views: layer1 [4,128] at cols 512..639, layers 2..5
        # [128,128] at cols 0..511, layer6 [128,6] at 640..645
        w_sb = [wb_sb[0:4, 512:640]]
        for i in range(4):
            w_sb.append(wb_sb[:, i * 128 : (i + 1) * 128])
        w_sb.append(wb_sb[:, 640:646])
        b_sb = [bias_sb[:, i : i + 1] for i in range(5)]
        c0_sb = [bias_sb[:, 5 + i : 6 + i] for i in range(5)]

        st = {"l6": 0, "grp": None, "grp_blocks": [], "flush_i": 0}

        def flush_group():
            grp = st["grp"]
            gsb = gs_pool.tile([128, BLK], F16, tag="gs")
            if st["flush_i"] % 2 == 0:
                nc.scalar.copy(out=gsb[:, :], in_=grp[:, :])
            else:
                nc.vector.tensor_copy(gsb[:, :], grp[:, :])
            st["flush_i"] += 1
            for i, c0 in enumerate(st["grp_blocks"]):
                nc.gpsimd.dma_start(
                    out=out_t[:, c0 : c0 + BLK],
                    in_=gsb[32 * i : 32 * i + 6, :],
                )
            st["grp"] = None
            st["grp_blocks"] = []

        def emit_step(chain, step):
            b = step // 6
            layer = step % 6
            blk = chain["blocks"][b]
            c0 = blk * BLK
            h = chain.get("h")
            if layer == 5:
                k = st["l6"] % GRP
                if k == 0:
                    grp_t = pg_pool.tile([128, BLK], F32, tag="grp", name="grp")
                    st["grp"] = grp_t
                grp = st["grp"]
                for q in range(BLK // MM):
                    nc.tensor.matmul(
                        out=grp[32 * k : 32 * k + 6, q * MM : (q + 1) * MM],
                        lhsT=w_sb[5],
                        rhs=h[:, q * MM : (q + 1) * MM],
                        start=True, stop=True,
                        tile_position=(0, 32 * k),
                    )
                st["grp_blocks"].append(c0)
                st["l6"] += 1
                if k == GRP - 1:
                    flush_group()
                return
            ps = ps_pool.tile([128, BLK], F32, tag="ps")
            for q in range(BLK // MM):
                if layer == 0:
                    rhs = xfull[:, c0 + q * MM : c0 + (q + 1) * MM]
                else:
                    rhs = h[:, q * MM : (q + 1) * MM]
                nc.tensor.matmul(
                    out=ps[:, q * MM : (q + 1) * MM],
                    lhsT=w_sb[layer], rhs=rhs,
                    start=True, stop=True,
                )
            hn = h_pool.tile([128, BLK], F16, tag=f"h{chain['id']}")
            if chain["kind"] == "A" or layer == 0:
                # layer 1 always runs on ACT: its preactivation range is the
                # widest, so exact tanh there buys the most accuracy.
                nc.scalar.activation(
                    out=hn[:, :], in_=ps[:, :],
                    func=mybir.ActivationFunctionType.Tanh,
                    bias=b_sb[layer],
                )
            else:
                c = coeffs[layer]
                nc.vector._custom_dve(
                    TANH5, out=hn[:, :], in0=ps[:, :],
                    in1=c0_sb[layer], s0=b_sb[layer],
                    s1=float(c[2]), imm2=float(c[1]),
                )
            chain["h"] = hn

        for i, ch in enumerate(chains):
            ch["id"] = i

        for rep in range(repeat):
            live = []
            for i, ch in enumerate(chains):
                ch["step"] = 0
                ch["vt"] = i * 280.0
                ch["nsteps"] = len(ch["blocks"]) * 6
                if ch["nsteps"]:
                    live.append(ch)
            while live:
                ch = min(live, key=lambda c: c["vt"])
                emit_step(ch, ch["step"])
                ch["vt"] += step_cost(ch, ch["step"])
                ch["step"] += 1
                if ch["step"] >= ch["nsteps"]:
                    live.remove(ch)
            if st["grp_blocks"]:
                flush_group()

    nc.compile()
    return nc


# ---------------------------------------------------------------------------
# Host-side weight prep
# ---------------------------------------------------------------------------

def _eff(w, bmat, amat):
    return (
        w.astype(np.float64) + bmat.astype(np.float64) @ amat.astype(np.float64)
    ).astype(np.float32)


def _prep(inputs):
    Weff = [_eff(inputs[f"W{i}"], inputs[f"B{i}"], inputs[f"A{i}"])
            for i in range(1, 7)]
    beff = [np.asarray(inputs[f"b{i}"], np.float32).reshape(-1)
            for i in range(1, 7)]

    wb = np.zeros((128, WB_COLS), np.float16)
    for i in (2, 3, 4, 5):
        wl = Weff[i - 1]
        c = (i - 2) * 128
        wb[0:64, c : c + 64] = wl.T.astype(np.float16)
        wb[64:128, c + 64 : c + 128] = wl.T.astype(np.float16)
    w1 = Weff[0]  # [64, 2]
    wb[0:2, 512:576] = w1.T.astype(np.float16)
    wb[2:4, 576:640] = w1.T.astype(np.float16)
    w6 = Weff[5]  # [3, 64]
    wb[0:64, 640:643] = w6.T.astype(np.float16)
    wb[64:128, 643:646] = w6.T.astype(np.float16)

    coeffs = _poly_coeffs(inputs, Weff, beff)

    biasc = np.zeros((128, 10), np.float32)
    for i in range(5):
        b = beff[i]
        biasc[:, i] = np.concatenate([b, b])
        biasc[:, 5 + i] = coeffs[i][0]
    return {"wb": wb, "biasc": biasc}, coeffs, beff[5]


def _in_maps(inputs, ws):
    x = np.asarray(inputs["x"], np.float32)
    maps = []
    for c in range(NCORES):
        sh = x[c * N_CORE : (c + 1) * N_CORE]  # [131072, 2]
        xtc = np.empty((4, NCOLS), np.float16)
        xtc[0:2] = sh[:NCOLS].T
        xtc[2:4] = sh[NCOLS:].T
        m = {"xt": np.ascontiguousarray(xtc)}
        m.update(ws)
        maps.append(m)
    return maps


def kernel(**inputs):
    global LAST_RESULT
    inputs = {k: np.asarray(v, np.float32) for k, v in inputs.items()}
    ws, coeffs, b6 = _prep(inputs)
    maps = _in_maps(inputs, ws)

    nc = build_nc(coeffs)
    res = run_bass_kernel_spmd(nc, maps, core_ids=list(range(NCORES)))
    LAST_RESULT = res

    u = np.empty((N, 1), np.float32)
    v = np.empty((N, 1), np.float32)
    w = np.empty((N, 1), np.float32)
    for c in range(NCORES):
        o = res.results[c]["out_t"].astype(np.float32)  # [6, NCOLS]
        base = c * N_CORE
        u[base : base + NCOLS, 0] = o[0] + b6[0]
        v[base : base + NCOLS, 0] = o[1] + b6[1]
        w[base : base + NCOLS, 0] = o[2] + b6[2]
        u[base + NCOLS : base + N_CORE, 0] = o[3] + b6[0]
        v[base + NCOLS : base + N_CORE, 0] = o[4] + b6[1]
        w[base + NCOLS : base + N_CORE, 0] = o[5] + b6[2]
    return (u, v, w)


def measure_exec_ns(r=17, rounds=12):
    """Per-execution HW time via paired repeat-delta (drift-immune)."""
    import time as _time

    import jax
    from jax.sharding import Mesh, PartitionSpec
    from jax.experimental.shard_map import shard_map

    from concourse.bass2jax import (
        _bass_exec_p,
        install_neuronx_cc_hook,
        partition_id_tensor,
    )

    z_in = np.load("ref_cache.npz")
    inputs = {k[3:]: np.asarray(z_in[k], np.float32)
              for k in z_in.files if k.startswith("in_")}
    ws, coeffs, _b6 = _prep(inputs)
    maps = _in_maps(inputs, ws)

    def make_fn(nc):
        install_neuronx_cc_hook()
        in_names, out_names, out_avals = [], [], []
        for alloc in nc.m.functions[0].allocations:
            if not isinstance(alloc, mybir.MemoryLocationSet):
                continue
            name = alloc.memorylocations[0].name
            if alloc.kind == "ExternalInput":
                in_names.append(name)
            elif alloc.kind == "ExternalOutput":
                out_names.append(name)
                out_avals.append(jax.core.ShapedArray(
                    tuple(alloc.tensor_shape), mybir.dt.np(alloc.dtype)))
        pname = nc.partition_id_tensor.name if nc.partition_id_tensor else None
        if pname in in_names:
            in_names.remove(pname)
        all_in = in_names + out_names + ([pname] if pname else [])

        def _body(*flat):
            extra = (partition_id_tensor(),) if pname else ()
            return tuple(_bass_exec_p.bind(
                *flat, *extra, out_avals=tuple(out_avals),
                in_names=tuple(all_in), out_names=tuple(out_names),
                lowering_input_output_aliases=(), sim_require_finite=True,
                sim_require_nnan=True, nc=nc))

        mesh = Mesh(np.asarray(jax.devices()[:NCORES]), ("core",))
        specs = (PartitionSpec("core"),) * (len(in_names) + len(out_names))
        f = jax.jit(shard_map(_body, mesh=mesh, in_specs=specs,
                    out_specs=(PartitionSpec("core"),) * len(out_names),
                    check_rep=False), keep_unused=True)
        return f, in_names

    mesh = Mesh(np.asarray(jax.devices()[:NCORES]), ("core",))
    sharding = jax.sharding.NamedSharding(mesh, PartitionSpec("core"))
    variants = []
    for rep in (1, r):
        f, in_names = make_fn(build_nc(coeffs, repeat=rep))
        per_core = [[np.asarray(m[nm]) for nm in in_names] for m in maps]
        concat = [np.concatenate([per_core[c][i] for c in range(NCORES)], axis=0)
                  for i in range(len(in_names))]
        concat.append(np.zeros((NCORES * 6, NCOLS), np.float16))
        dev = [jax.device_put(a, sharding) for a in concat]
        jax.block_until_ready(dev)
        jax.block_until_ready(f(*dev))
        variants.append((f, dev))
    deltas = []
    for _ in range(rounds):
        ts = []
        for f, dev in variants:
            t0 = _time.time()
            jax.block_until_ready(f(*dev))
            ts.append(_time.time() - t0)
        deltas.append(ts[1] - ts[0])
    deltas.sort()
    return deltas[len(deltas) // 2] / (r - 1) * 1e9


# revision 11
# speedup vs baseline: 2.1323x; 1.1651x over previous
"""LoRA MLP (2->64->64->64->64->64->3, tanh) over N=1,048,576 rows.

Strategy (v2):
  - Host: merge LoRA into dense weights (W_eff = W + B@A), build
    block-diagonal lhsT so each 128-wide PE pass processes TWO row-chunks
    (features of chunk A on partitions 0..63, chunk B on 64..127).
  - 8 cores, pure data parallel: 131072 rows/core = 65536 columns.
  - tanh is split across BOTH the scalar (ACT) engine and the vector
    (DVE) engine: a custom fused DVE op evaluates a per-layer degree-5
    odd minimax polynomial x*(c0 + c1 x^2 + c2 x^4) (with the fused
    per-partition bias add) in ONE DVE instruction at 1 elem/lane/cyc,
    nearly matching ACT's tanh throughput. Blocks of 1024 columns are
    statically assigned to ACT-chains or DVE-chains so both engines run
    ~100% busy; the PE (6 matmul passes) runs just below them.
  - Layer-6 outputs ([6,1024] per block) are batched 4 blocks per PSUM
    tile at PE quadrant offsets 0/32/64/96, then flushed with a single
    [128,1024] copy (alternating ACT/DVE) -> SBUF -> DMA. b6 is added
    on the host (free).
  - PSUM: 3x [128,1024] main tiles (6 banks) + 1x [128,1024] group tile
    (2 banks) = 8 banks.
  - Polynomial coefficients are fit at runtime from a sampled forward
    pass (per-layer preactivation range), so accuracy tracks the data.
"""

import os
import numpy as np
from contextlib import ExitStack

import concourse.bacc as bacc
import concourse.tile as tile
from concourse import mybir
from concourse.bass_utils import run_bass_kernel_spmd

import concourse.dve_ops as _dve_ops
from concourse.dve_spec import (
    Spec, Src0, C0, C1, C2, C3, sq, lower, _spill_c3_to_src1,
)
from concourse.dve_ops import DveOp, OPS, CUSTOM_DVE_SPECS
from concourse.dve_uop import DveOpSpec

N = 1_048_576
NCORES = 8
N_CORE = N // NCORES          # 131072 rows per core
NCOLS = N_CORE // 2           # 65536 cols (2 rows per col: chunk A + chunk B)
BLK = 1024                    # columns per block (psum tile = 2 banks)
NBLK = NCOLS // BLK           # 64 blocks
MM = 512                      # moving free dim per matmul (1 PSUM bank)
N_D = int(os.environ.get("BASS_ND", "39"))  # blocks w/ layers 2-5 on DVE poly
GRP = 4                       # layer-6 outputs grouped per psum tile

F32 = mybir.dt.float32
F16 = mybir.dt.float16

# Set by the last kernel() call (profiling info for test.py).
LAST_RESULT = None


# ---------------------------------------------------------------------------
# Custom fused DVE op: out = (((c2*t + c1)*t + c0) * x, x = in0 + bias,
# t = x*x.  One DVE instruction per [128, BLK] tile (1 elem/lane/cyc).
# ---------------------------------------------------------------------------

def _tanh5_ref(in0, in1, s0, s1, imm2):
    x = in0.astype(np.float32) + s0
    t = x * x
    return ((s1 * t + imm2) * t + in1) * x


def _register_tanh5():
    for op in OPS:
        if op.name == "TANH5_ANT":
            return op
    _x = Src0 + C0
    _t = sq(_x)
    body = _spill_c3_to_src1((((C1 * _t) + C2) * _t + C3) * _x)
    spec = Spec(body=body, reference=_tanh5_ref)
    shas = {}
    for ver in ("v3", "v4"):
        shas[ver] = DveOpSpec(
            name="TANH5_ANT", opcode=1 + len(OPS),
            uops=lower(spec, ver=ver), rd1_en=True,
        ).sha(ver)
    op = DveOp("TANH5_ANT", spec, subdim=False, uops_sha=shas)
    OPS.append(op)
    CUSTOM_DVE_SPECS[op.name] = op.spec
    _dve_ops._SUB_OPCODE_FOR_NAME[op.name] = OPS.index(op) + 1
    return op


TANH5 = _register_tanh5()


# ---------------------------------------------------------------------------
# Per-layer degree-5 odd minimax fit of tanh on [0, B] (Lawson iteration).
# ---------------------------------------------------------------------------

def _fit_tanh5(B, n=4001, iters=40):
    xs = np.linspace(0.0, B, n)
    y = np.tanh(xs)
    A = np.stack([xs, xs**3, xs**5], axis=1)
    w = np.ones(n)
    c = None
    for _ in range(iters):
        Aw = A * w[:, None]
        c, *_ = np.linalg.lstsq(Aw, y * w, rcond=None)
        e = np.abs(A @ c - y)
        w *= e + 1e-12
        w /= w.max()
    return c.astype(np.float64)  # (c0, c1, c2)


def _poly_coeffs(inputs, Weff, beff):
    """Sampled forward pass -> per-layer preactivation range -> coeffs."""
    xs = np.asarray(inputs["x"], np.float32)[::16].astype(np.float32)
    h = xs
    coeffs = []
    for l in range(5):
        z = h @ Weff[l].T.astype(np.float32) + beff[l].astype(np.float32)
        B = float(np.abs(z).max()) * 1.08 + 0.03
        c = _fit_tanh5(B)
        coeffs.append(c)
        h = np.tanh(z)
    return coeffs  # list of (c0, c1, c2)


# ---------------------------------------------------------------------------
# Kernel build
# ---------------------------------------------------------------------------

WB_COLS = 646  # packed fp16 lhsT weights


def _d_blocks():
    picks = sorted({int(round(i * NBLK / N_D)) for i in range(N_D)})
    # de-dup & fill to exactly N_D entries
    out, used = [], set()
    for p in picks:
        while p in used:
            p += 1
        p %= NBLK
        while p in used:
            p = (p + 1) % NBLK
        used.add(p)
        out.append(p)
    return sorted(out)


def build_nc(coeffs, repeat=1):
    nc = bacc.Bacc(None, target_bir_lowering=False)

    xt = nc.dram_tensor("xt", [4, NCOLS], F16, kind="ExternalInput")
    wb = nc.dram_tensor("wb", [128, WB_COLS], F16, kind="ExternalInput")
    biasc = nc.dram_tensor("biasc", [128, 10], F32, kind="ExternalInput")
    out_t = nc.dram_tensor("out_t", [6, NCOLS], F16, kind="ExternalOutput")

    dset = set(_d_blocks())
    # Deal blocks round-robin to 4 chains. Each chain carries a similar mix
    # of A-blocks (tanh on ACT) and D-blocks (poly on DVE), so all chains
    # have equal total duration — no engine-starved tail.
    chains = [{"blocks": list(range(c, NBLK, 4))} for c in range(4)]
    CA, CD, CL6 = 900.0, 950.0, 450.0

    def blk_kind(blk):
        return "D" if blk in dset else "A"

    def step_cost(chain, step):
        layer = step % 6
        blk = chain["blocks"][step // 6]
        if layer == 5:
            return CL6
        if blk_kind(blk) == "A" or layer == 0:
            return CA
        return CD

    with tile.TileContext(nc) as tc, ExitStack() as ctx:
        const = ctx.enter_context(tc.tile_pool(name="const", bufs=1))
        h_pool = ctx.enter_context(tc.tile_pool(name="h", bufs=3))
        gs_pool = ctx.enter_context(tc.tile_pool(name="gs", bufs=2))
        ps_pool = ctx.enter_context(tc.tile_pool(name="ps", bufs=3, space="PSUM"))
        pg_pool = ctx.enter_context(tc.tile_pool(name="pg", bufs=1, space="PSUM"))

        wb_sb = const.tile([128, WB_COLS], F16, tag="wb")
        nc.gpsimd.dma_start(out=wb_sb, in_=wb[:, :])
        bias_sb = const.tile([128, 10], F32, tag="biasc")
        nc.gpsimd.dma_start(out=bias_sb, in_=biasc[:, :])

        xfull = const.tile([4, NCOLS], F16, tag="xfull")
        XCH = NCOLS // 8
        for ch in range(8):
            nc.gpsimd.dma_start(
                out=xfull[:, ch * XCH : (ch + 1) * XCH],
                in_=xt[:, ch * XCH : (ch + 1) * XCH],
            )

        # lhsT views: layer1 [4,128] at cols 512..639, layers 2..5
        # [128,128] at cols 0..511, layer6 [128,6] at 640..645
        w_sb = [wb_sb[0:4, 512:640]]
        for i in range(4):
            w_sb.append(wb_sb[:, i * 128 : (i + 1) * 128])
        w_sb.append(wb_sb[:, 640:646])
        b_sb = [bias_sb[:, i : i + 1] for i in range(5)]
        c0_sb = [bias_sb[:, 5 + i : 6 + i] for i in range(5)]

        st = {"l6": 0, "grp": None, "grp_blocks": [], "flush_i": 0}

        def flush_group():
            grp = st["grp"]
            gsb = gs_pool.tile([128, BLK], F16, tag="gs")
            if st["flush_i"] % 2 == 0:
                nc.scalar.copy(out=gsb[:, :], in_=grp[:, :])
            else:
                nc.vector.tensor_copy(gsb[:, :], grp[:, :])
            st["flush_i"] += 1
            for i, c0 in enumerate(st["grp_blocks"]):
                nc.gpsimd.dma_start(
                    out=out_t[:, c0 : c0 + BLK],
                    in_=gsb[32 * i : 32 * i + 6, :],
                )
            st["grp"] = None
            st["grp_blocks"] = []

        def emit_step(chain, step):
            b = step // 6
            layer = step % 6
            blk = chain["blocks"][b]
            c0 = blk * BLK
            h = chain.get("h")
            if layer == 5:
                k = st["l6"] % GRP
                if k == 0:
                    grp_t = pg_pool.tile([128, BLK], F32, tag="grp", name="grp")
                    st["grp"] = grp_t
                grp = st["grp"]
                for q in range(BLK // MM):
                    nc.tensor.matmul(
                        out=grp[32 * k : 32 * k + 6, q * MM : (q + 1) * MM],
                        lhsT=w_sb[5],
                        rhs=h[:, q * MM : (q + 1) * MM],
                        start=True, stop=True,
                        tile_position=(0, 32 * k),
                    )
                st["grp_blocks"].append(c0)
                st["l6"] += 1
                if k == GRP - 1:
                    flush_group()
                return
            ps = ps_pool.tile([128, BLK], F32, tag="ps")
            for q in range(BLK // MM):
                if layer == 0:
                    rhs = xfull[:, c0 + q * MM : c0 + (q + 1) * MM]
                else:
                    rhs = h[:, q * MM : (q + 1) * MM]
                nc.tensor.matmul(
                    out=ps[:, q * MM : (q + 1) * MM],
                    lhsT=w_sb[layer], rhs=rhs,
                    start=True, stop=True,
                )
            hn = h_pool.tile([128, BLK], F16, tag=f"h{chain['id']}")
            if blk_kind(blk) == "A" or layer == 0:
                # layer 1 always runs on ACT: its preactivation range is the
                # widest, so exact tanh there buys the most accuracy.
                nc.scalar.activation(
                    out=hn[:, :], in_=ps[:, :],
                    func=mybir.ActivationFunctionType.Tanh,
                    bias=b_sb[layer],
                )
            else:
                c = coeffs[layer]
                nc.vector._custom_dve(
                    TANH5, out=hn[:, :], in0=ps[:, :],
                    in1=c0_sb[layer], s0=b_sb[layer],
                    s1=float(c[2]), imm2=float(c[1]),
                )
            chain["h"] = hn

        for i, ch in enumerate(chains):
            ch["id"] = i

        for rep in range(repeat):
            live = []
            for i, ch in enumerate(chains):
                ch["step"] = 0
                ch["vt"] = i * 280.0
                ch["nsteps"] = len(ch["blocks"]) * 6
                if ch["nsteps"]:
                    live.append(ch)
            while live:
                ch = min(live, key=lambda c: c["vt"])
                emit_step(ch, ch["step"])
                ch["vt"] += step_cost(ch, ch["step"])
                ch["step"] += 1
                if ch["step"] >= ch["nsteps"]:
                    live.remove(ch)
            if st["grp_blocks"]:
                flush_group()

    nc.compile()
    return nc


# ---------------------------------------------------------------------------
# Host-side weight prep
# ---------------------------------------------------------------------------

def _eff(w, bmat, amat):
    return (
        w.astype(np.float64) + bmat.astype(np.float64) @ amat.astype(np.float64)
    ).astype(np.float32)


def _prep(inputs):
    Weff = [_eff(inputs[f"W{i}"], inputs[f"B{i}"], inputs[f"A{i}"])
            for i in range(1, 7)]
    beff = [np.asarray(inputs[f"b{i}"], np.float32).reshape(-1)
            for i in range(1, 7)]

    wb = np.zeros((128, WB_COLS), np.float16)
    for i in (2, 3, 4, 5):
        wl = Weff[i - 1]
        c = (i - 2) * 128
        wb[0:64, c : c + 64] = wl.T.astype(np.float16)
        wb[64:128, c + 64 : c + 128] = wl.T.astype(np.float16)
    w1 = Weff[0]  # [64, 2]
    wb[0:2, 512:576] = w1.T.astype(np.float16)
    wb[2:4, 576:640] = w1.T.astype(np.float16)
    w6 = Weff[5]  # [3, 64]
    wb[0:64, 640:643] = w6.T.astype(np.float16)
    wb[64:128, 643:646] = w6.T.astype(np.float16)

    coeffs = _poly_coeffs(inputs, Weff, beff)

    biasc = np.zeros((128, 10), np.float32)
    for i in range(5):
        b = beff[i]
        biasc[:, i] = np.concatenate([b, b])
        biasc[:, 5 + i] = coeffs[i][0]
    return {"wb": wb, "biasc": biasc}, coeffs, beff[5]


def _in_maps(inputs, ws):
    x = np.asarray(inputs["x"], np.float32)
    maps = []
    for c in range(NCORES):
        sh = x[c * N_CORE : (c + 1) * N_CORE]  # [131072, 2]
        xtc = np.empty((4, NCOLS), np.float16)
        xtc[0:2] = sh[:NCOLS].T
        xtc[2:4] = sh[NCOLS:].T
        m = {"xt": np.ascontiguousarray(xtc)}
        m.update(ws)
        maps.append(m)
    return maps


def kernel(**inputs):
    global LAST_RESULT
    inputs = {k: np.asarray(v, np.float32) for k, v in inputs.items()}
    ws, coeffs, b6 = _prep(inputs)
    maps = _in_maps(inputs, ws)

    nc = build_nc(coeffs)
    res = run_bass_kernel_spmd(nc, maps, core_ids=list(range(NCORES)))
    LAST_RESULT = res

    u = np.empty((N, 1), np.float32)
    v = np.empty((N, 1), np.float32)
    w = np.empty((N, 1), np.float32)
    for c in range(NCORES):
        o = res.results[c]["out_t"].astype(np.float32)  # [6, NCOLS]
        base = c * N_CORE
        u[base : base + NCOLS, 0] = o[0] + b6[0]
        v[base : base + NCOLS, 0] = o[1] + b6[1]
        w[base : base + NCOLS, 0] = o[2] + b6[2]
        u[base + NCOLS : base + N_CORE, 0] = o[3] + b6[0]
        v[base + NCOLS : base + N_CORE, 0] = o[4] + b6[1]
        w[base + NCOLS : base + N_CORE, 0] = o[5] + b6[2]
    return (u, v, w)


def measure_exec_ns(r=17, rounds=12):
    """Per-execution HW time via paired repeat-delta (drift-immune)."""
    import time as _time

    import jax
    from jax.sharding import Mesh, PartitionSpec
    from jax.experimental.shard_map import shard_map

    from concourse.bass2jax import (
        _bass_exec_p,
        install_neuronx_cc_hook,
        partition_id_tensor,
    )

    z_in = np.load("ref_cache.npz")
    inputs = {k[3:]: np.asarray(z_in[k], np.float32)
              for k in z_in.files if k.startswith("in_")}
    ws, coeffs, _b6 = _prep(inputs)
    maps = _in_maps(inputs, ws)

    def make_fn(nc):
        install_neuronx_cc_hook()
        in_names, out_names, out_avals = [], [], []
        for alloc in nc.m.functions[0].allocations:
            if not isinstance(alloc, mybir.MemoryLocationSet):
                continue
            name = alloc.memorylocations[0].name
            if alloc.kind == "ExternalInput":
                in_names.append(name)
            elif alloc.kind == "ExternalOutput":
                out_names.append(name)
                out_avals.append(jax.core.ShapedArray(
                    tuple(alloc.tensor_shape), mybir.dt.np(alloc.dtype)))
        pname = nc.partition_id_tensor.name if nc.partition_id_tensor else None
        if pname in in_names:
            in_names.remove(pname)
        all_in = in_names + out_names + ([pname] if pname else [])

        def _body(*flat):
            extra = (partition_id_tensor(),) if pname else ()
            return tuple(_bass_exec_p.bind(
                *flat, *extra, out_avals=tuple(out_avals),
                in_names=tuple(all_in), out_names=tuple(out_names),
                lowering_input_output_aliases=(), sim_require_finite=True,
                sim_require_nnan=True, nc=nc))

        mesh = Mesh(np.asarray(jax.devices()[:NCORES]), ("core",))
        specs = (PartitionSpec("core"),) * (len(in_names) + len(out_names))
        f = jax.jit(shard_map(_body, mesh=mesh, in_specs=specs,
                    out_specs=(PartitionSpec("core"),) * len(out_names),
                    check_rep=False), keep_unused=True)
        return f, in_names

    mesh = Mesh(np.asarray(jax.devices()[:NCORES]), ("core",))
    sharding = jax.sharding.NamedSharding(mesh, PartitionSpec("core"))
    variants = []
    for rep in (1, r):
        f, in_names = make_fn(build_nc(coeffs, repeat=rep))
        per_core = [[np.asarray(m[nm]) for nm in in_names] for m in maps]
        concat = [np.concatenate([per_core[c][i] for c in range(NCORES)], axis=0)
                  for i in range(len(in_names))]
        concat.append(np.zeros((NCORES * 6, NCOLS), np.float16))
        dev = [jax.device_put(a, sharding) for a in concat]
        jax.block_until_ready(dev)
        jax.block_until_ready(f(*dev))
        variants.append((f, dev))
    deltas = []
    for _ in range(rounds):
        ts = []
        for f, dev in variants:
            t0 = _time.time()
            jax.block_until_ready(f(*dev))
            ts.append(_time.time() - t0)
        deltas.append(ts[1] - ts[0])
    deltas.sort()
    return deltas[len(deltas) // 2] / (r - 1) * 1e9
